# revision 1
# baseline (speedup 1.0000x reference)
"""Grouped SwiGLU expert FFN (MoE) on 8 Trainium2 NeuronCores.

Expert parallelism: expert e's weights + its (pre-sorted) token slice go to
core e. Each core runs x@w1, x@w3, silu/mul, h@w2 for its 8192 tokens.

Math per core (dims: t=tokens, i=dim_in, j=dim_hid, o=dim_in):
  mm1/mm3: psum[j,t] += w{1,3}[i,j].T-style: lhsT=w[i_chunk, j_chunk] (natural
           layout, stationary), rhs=xT[i_chunk, t_block] (moving) -> h1T/h3T.
           Requires x transposed; done on-chip via PE transpose.
  SwiGLU:  hT = silu(h1T) * h3T  (ACT Silu + DVE mul, PSUM eviction fused).
  mm2:     lhsT=hT[j_chunk, t_chunk] (stationary), rhs=w2[j_chunk, o_block]
           (moving) -> psum[t,o] = natural-layout output. No output transpose.

All matmuls run in float32r (full PE rate at moving dim >= 256, ~1.5e-4 rel
err vs 2.3e-3 for bf16 -- measured on HW).
"""

import sys

sys.path.insert(0, "/opt/trn_rl_repo")

import numpy as np

N_CORES = 8
D = 1024  # dim_in
H = 1024  # dim_hid
P = 128
TB = 256  # token block per pipeline stage

_CACHE = {}


def _build(tok):
    import concourse.bacc as bacc
    import concourse.tile as tile
    from concourse import mybir
    from concourse.masks import make_identity

    dt = mybir.dt
    AF = mybir.ActivationFunctionType
    f32 = dt.float32
    f32r = dt.float32r

    assert tok % TB == 0
    n_blk = tok // TB
    n_i = D // P   # 8 contraction chunks for mm1/mm3
    n_j = H // P   # 8 contraction chunks for mm2
    n_tc = TB // P  # 2 token chunks per block
    n_o = D // 512  # 2 output column blocks

    nc = bacc.Bacc(trn_type="TRN2", target_bir_lowering=False)
    x_h = nc.dram_tensor("x", [tok, D], f32, kind="ExternalInput")
    w1_h = nc.dram_tensor("w1", [D, H], f32, kind="ExternalInput")
    w2_h = nc.dram_tensor("w2", [H, D], f32, kind="ExternalInput")
    w3_h = nc.dram_tensor("w3", [D, H], f32, kind="ExternalInput")
    out_h = nc.dram_tensor("out", [tok, D], f32, kind="ExternalOutput")

    with tile.TileContext(nc) as tc:
        with (
            tc.tile_pool(name="wpool", bufs=1) as wpool,
            tc.tile_pool(name="const", bufs=1) as const,
            tc.tile_pool(name="xpool", bufs=2) as xpool,
            tc.tile_pool(name="xtpool", bufs=2) as xtpool,
            tc.tile_pool(name="htpool", bufs=2) as htpool,
            tc.tile_pool(name="spool", bufs=3) as spool,
            tc.tile_pool(name="opool", bufs=2) as opool,
            tc.tile_pool(name="pT", bufs=2, space="PSUM") as pTp,
            tc.tile_pool(name="pA", bufs=2, space="PSUM") as pAp,
            tc.tile_pool(name="pB", bufs=2, space="PSUM") as pBp,
            tc.tile_pool(name="pC", bufs=2, space="PSUM") as pCp,
        ):
            ident = const.tile([P, P], f32)
            make_identity(nc, ident)

            # Resident weights, partition = row-within-chunk: [P, n_chunks, cols]
            w1s = wpool.tile([P, n_i, H], f32r)
            w3s = wpool.tile([P, n_i, H], f32r)
            w2s = wpool.tile([P, n_j, D], f32r)
            nc.sync.dma_start(
                out=w1s, in_=w1_h[:, :].rearrange("(c p) h -> p c h", p=P).bitcast(f32r)
            )
            nc.sync.dma_start(
                out=w3s, in_=w3_h[:, :].rearrange("(c p) h -> p c h", p=P).bitcast(f32r)
            )
            nc.sync.dma_start(
                out=w2s, in_=w2_h[:, :].rearrange("(c p) h -> p c h", p=P).bitcast(f32r)
            )

            x_r = x_h[:, :].rearrange("(b c p) d -> b p c d", p=P, c=n_tc)
            o_r = out_h[:, :].rearrange("(b c p) d -> b p c d", p=P, c=n_tc)

            for b in range(n_blk):
                # ---- load x block, natural layout [P, n_tc, D]
                x_sb = xpool.tile([P, n_tc, D], f32)
                nc.sync.dma_start(out=x_sb, in_=x_r[b])

                # ---- PE-transpose into xT [P(=i in chunk), n_i, TB] f32r
                xT = xtpool.tile([P, n_i, TB], f32r)
                for t in range(n_tc):
                    for i in range(n_i):
                        pT = pTp.tile([P, P], f32)
                        nc.tensor.transpose(
                            pT, x_sb[:, t, i * P:(i + 1) * P], ident
                        )
                        nc.scalar.activation(
                            xT[:, i, t * P:(t + 1) * P], pT, AF.Copy
                        )

                # ---- mm1/mm3 + SwiGLU -> hT [P(=j in chunk), n_j, TB] f32r
                hT = htpool.tile([P, n_j, TB], f32r)
                for j in range(n_j):
                    pA = pAp.tile([P, TB], f32)
                    pB = pBp.tile([P, TB], f32)
                    for i in range(n_i):
                        nc.tensor.matmul(
                            pA, w1s[:, i, j * P:(j + 1) * P], xT[:, i, :],
                            start=(i == 0), stop=(i == n_i - 1),
                        )
                    for i in range(n_i):
                        nc.tensor.matmul(
                            pB, w3s[:, i, j * P:(j + 1) * P], xT[:, i, :],
                            start=(i == 0), stop=(i == n_i - 1),
                        )
                    s1 = spool.tile([P, TB], f32)
                    nc.scalar.activation(s1, pA, AF.Silu)
                    nc.vector.tensor_mul(hT[:, j, :], pB, s1)

                # ---- mm2 -> natural-layout out block
                o_sb = opool.tile([P, n_tc, D], f32)
                for t in range(n_tc):
                    for o in range(n_o):
                        pC = pCp.tile([P, 512], f32)
                        for j in range(n_j):
                            nc.tensor.matmul(
                                pC,
                                hT[:, j, t * P:(t + 1) * P],
                                w2s[:, j, o * 512:(o + 1) * 512],
                                start=(j == 0), stop=(j == n_j - 1),
                            )
                        nc.scalar.activation(
                            o_sb[:, t, o * 512:(o + 1) * 512], pC, AF.Copy
                        )
                nc.sync.dma_start(out=o_r[b], in_=o_sb)

    nc.compile()
    return nc


def _get_nc(tok):
    if tok not in _CACHE:
        _CACHE[tok] = _build(tok)
    return _CACHE[tok]


def kernel(x, w1, w2, w3, m_sizes):
    from concourse.bass_utils import run_bass_kernel_spmd

    x = np.asarray(x, dtype=np.float32)
    w1 = np.asarray(w1, dtype=np.float32)
    w2 = np.asarray(w2, dtype=np.float32)
    w3 = np.asarray(w3, dtype=np.float32)
    sizes = np.asarray(m_sizes).astype(np.int64)
    offs = np.concatenate([[0], np.cumsum(sizes)])
    n_exp = sizes.shape[0]
    assert n_exp == N_CORES

    pad = int(max(int(sizes.max()), TB))
    pad = ((pad + TB - 1) // TB) * TB
    nc = _get_nc(pad)

    in_maps = []
    for e in range(N_CORES):
        xe = x[offs[e]:offs[e + 1]]
        if xe.shape[0] < pad:
            xe = np.concatenate(
                [xe, np.zeros((pad - xe.shape[0], D), dtype=np.float32)], axis=0
            )
        in_maps.append({"x": xe, "w1": w1[e], "w2": w2[e], "w3": w3[e]})

    r = run_bass_kernel_spmd(nc, in_maps, core_ids=list(range(N_CORES)))
    out = np.concatenate(
        [r.results[e]["out"][: sizes[e]] for e in range(N_CORES)], axis=0
    )
    return out.astype(np.float32)



# revision 2
# speedup vs baseline: 1.0717x; 1.0717x over previous
"""Grouped SwiGLU expert FFN (MoE) on 8 Trainium2 NeuronCores.

Expert parallelism: expert e's weights + its (pre-sorted) token slice go to
core e. Each core runs x@w1, x@w3, silu/mul, h@w2 for its 8192 tokens.

v2: fp16 datapath (PE runs fp16 at 1 cycle/row like f32r, but transposes are
1-pass instead of f32's LOW_HIGH 2-pass, and SBUF traffic halves).
 - x and w1/w2/w3 are cast f32->fp16 in-flight by SWDGE (gpsimd) DMAs.
 - 512-token blocks: mm1/mm3 moving dim 512 -> half the PE instruction count
   of the 256-block f32r version.
 - w1 is loaded first so block-0 matmuls start as early as possible.

Math per core (dims: t=tokens, i=dim_in, j=dim_hid, o=dim_in):
  transpose: x16[t,i] -> xT[i,t] via PE identity matmul (fp16, 1 cyc/row).
  mm1/mm3: psum[j,t] += lhsT=w{1,3}[i_chunk, j_chunk] (stationary),
           rhs=xT[i_chunk, t_block] (moving 512) -> h1T/h3T.
  SwiGLU:  hT = silu(h1T) * h3T  (ACT Silu -> fp16, DVE mul -> fp16).
  mm2:     lhsT=hT[j_chunk, t_chunk] (stationary), rhs=w2[j_chunk, o_block]
           (moving 512) -> psum[t,o] natural-layout f32 output.
"""

import sys

sys.path.insert(0, "/opt/trn_rl_repo")

import numpy as np

N_CORES = 8
D = 1024  # dim_in
H = 1024  # dim_hid
P = 128
TB = 512  # token block per pipeline stage

_CACHE = {}


def _build(tok):
    import concourse.bacc as bacc
    import concourse.tile as tile
    from concourse import mybir
    from concourse.masks import make_identity

    dt = mybir.dt
    AF = mybir.ActivationFunctionType
    f32 = dt.float32
    f16 = dt.float16

    assert tok % TB == 0
    n_blk = tok // TB
    n_i = D // P   # 8 contraction chunks for mm1/mm3
    n_j = H // P   # 8 contraction chunks for mm2
    n_tc = TB // P  # 4 token chunks per block
    n_o = D // 512  # 2 output column blocks

    nc = bacc.Bacc(trn_type="TRN2", target_bir_lowering=False)
    x_h = nc.dram_tensor("x", [tok, D], f32, kind="ExternalInput")
    w1_h = nc.dram_tensor("w1", [D, H], f32, kind="ExternalInput")
    w2_h = nc.dram_tensor("w2", [H, D], f32, kind="ExternalInput")
    w3_h = nc.dram_tensor("w3", [D, H], f32, kind="ExternalInput")
    out_h = nc.dram_tensor("out", [tok, D], f32, kind="ExternalOutput")

    with tile.TileContext(nc) as tc:
        with (
            tc.tile_pool(name="wpool", bufs=1) as wpool,
            tc.tile_pool(name="const", bufs=1) as const,
            tc.tile_pool(name="xpool", bufs=3) as xpool,
            tc.tile_pool(name="xtpool", bufs=2) as xtpool,
            tc.tile_pool(name="htpool", bufs=2) as htpool,
            tc.tile_pool(name="spool", bufs=3) as spool,
            tc.tile_pool(name="opool", bufs=2) as opool,
            tc.tile_pool(name="pT", bufs=2, space="PSUM") as pTp,
            tc.tile_pool(name="pA", bufs=2, space="PSUM") as pAp,
            tc.tile_pool(name="pB", bufs=2, space="PSUM") as pBp,
            tc.tile_pool(name="pC", bufs=2, space="PSUM") as pCp,
        ):
            ident = const.tile([P, P], f16)
            make_identity(nc, ident)

            # Resident fp16 weights, cast in-flight by SWDGE cast-DMAs.
            # Layout: partition = row-within-chunk: [P, n_chunks, cols].
            w1s = wpool.tile([P, n_i, H], f16)
            w3s = wpool.tile([P, n_i, H], f16)
            w2s = wpool.tile([P, n_j, D], f16)
            nc.gpsimd.dma_start(
                out=w1s, in_=w1_h[:, :].rearrange("(c p) h -> p c h", p=P)
            )
            nc.gpsimd.dma_start(
                out=w3s, in_=w3_h[:, :].rearrange("(c p) h -> p c h", p=P)
            )
            nc.gpsimd.dma_start(
                out=w2s, in_=w2_h[:, :].rearrange("(c p) h -> p c h", p=P)
            )

            x_r = x_h[:, :].rearrange("(b c p) d -> b p c d", p=P, c=n_tc)
            o_r = out_h[:, :].rearrange("(b c p) d -> b p c d", p=P, c=n_tc)

            for b in range(n_blk):
                # ---- load + cast x block, natural layout [P, n_tc, D] fp16
                x16 = xpool.tile([P, n_tc, D], f16)
                nc.gpsimd.dma_start(out=x16, in_=x_r[b])

                # ---- PE-transpose into xT [P(=i in chunk), n_i, TB] fp16
                xT = xtpool.tile([P, n_i, TB], f16)
                for t in range(n_tc):
                    for i in range(n_i):
                        pT = pTp.tile([P, P], f16)
                        nc.tensor.transpose(
                            pT, x16[:, t, i * P:(i + 1) * P], ident
                        )
                        nc.scalar.activation(
                            xT[:, i, t * P:(t + 1) * P], pT, AF.Copy
                        )

                # ---- mm1/mm3 + SwiGLU -> hT [P(=j in chunk), n_j, TB] fp16
                hT = htpool.tile([P, n_j, TB], f16)
                for j in range(n_j):
                    pA = pAp.tile([P, TB], f32)
                    pB = pBp.tile([P, TB], f32)
                    for i in range(n_i):
                        nc.tensor.matmul(
                            pA, w1s[:, i, j * P:(j + 1) * P], xT[:, i, :],
                            start=(i == 0), stop=(i == n_i - 1),
                        )
                    for i in range(n_i):
                        nc.tensor.matmul(
                            pB, w3s[:, i, j * P:(j + 1) * P], xT[:, i, :],
                            start=(i == 0), stop=(i == n_i - 1),
                        )
                    s1 = spool.tile([P, TB], f16)
                    nc.scalar.activation(s1, pA, AF.Silu)
                    nc.vector.tensor_mul(hT[:, j, :], pB, s1)

                # ---- mm2 -> natural-layout out block
                o_sb = opool.tile([P, n_tc, D], f32)
                for t in range(n_tc):
                    for o in range(n_o):
                        pC = pCp.tile([P, 512], f32)
                        for j in range(n_j):
                            nc.tensor.matmul(
                                pC,
                                hT[:, j, t * P:(t + 1) * P],
                                w2s[:, j, o * 512:(o + 1) * 512],
                                start=(j == 0), stop=(j == n_j - 1),
                            )
                        nc.scalar.activation(
                            o_sb[:, t, o * 512:(o + 1) * 512], pC, AF.Copy
                        )
                nc.sync.dma_start(out=o_r[b], in_=o_sb)

    nc.compile()
    return nc


def _get_nc(tok):
    if tok not in _CACHE:
        _CACHE[tok] = _build(tok)
    return _CACHE[tok]


def kernel(x, w1, w2, w3, m_sizes):
    from concourse.bass_utils import run_bass_kernel_spmd

    x = np.asarray(x, dtype=np.float32)
    w1 = np.asarray(w1, dtype=np.float32)
    w2 = np.asarray(w2, dtype=np.float32)
    w3 = np.asarray(w3, dtype=np.float32)
    sizes = np.asarray(m_sizes).astype(np.int64)
    offs = np.concatenate([[0], np.cumsum(sizes)])
    n_exp = sizes.shape[0]
    assert n_exp == N_CORES

    pad = int(max(int(sizes.max()), TB))
    pad = ((pad + TB - 1) // TB) * TB
    nc = _get_nc(pad)

    in_maps = []
    for e in range(N_CORES):
        xe = x[offs[e]:offs[e + 1]]
        if xe.shape[0] < pad:
            xe = np.concatenate(
                [xe, np.zeros((pad - xe.shape[0], D), dtype=np.float32)], axis=0
            )
        in_maps.append({"x": xe, "w1": w1[e], "w2": w2[e], "w3": w3[e]})

    r = run_bass_kernel_spmd(nc, in_maps, core_ids=list(range(N_CORES)))
    out = np.concatenate(
        [r.results[e]["out"][: sizes[e]] for e in range(N_CORES)], axis=0
    )
    return out.astype(np.float32)


# revision 4
# speedup vs baseline: 1.2959x; 1.2093x over previous
"""Grouped SwiGLU expert FFN (MoE) on 8 Trainium2 NeuronCores.

Expert parallelism: expert e's weights + its (pre-sorted) token slice go to
core e. Each core runs x@w1, x@w3, silu/mul, h@w2 for its 8192 tokens.

v3: fp16 datapath (PE runs fp16 at 1 cycle/row like f32r, but transposes are
1-pass instead of f32's LOW_HIGH 2-pass, and SBUF traffic halves).
 - x and w1/w2/w3 are cast f32->fp16 in-flight by SWDGE (gpsimd) DMAs.
 - 512-token blocks: mm1/mm3 moving dim 512 -> half the PE instruction count
   of the 256-block f32r version.
 - x blocks 0-1 are DMA'd before the weights so the PE starts early.
 - transpose results are grouped 8-to-a-PSUM-bank and evicted with one wide
   copy, alternating ACT/DVE (v2 lost ~190us serializing 512 small evicts).

Math per core (dims: t=tokens, i=dim_in, j=dim_hid, o=dim_in):
  transpose: x16[t,i] -> xT[i,t] via PE identity matmul (fp16, 1 cyc/row).
  mm1/mm3: psum[j,t] += lhsT=w{1,3}[i_chunk, j_chunk] (stationary),
           rhs=xT[i_chunk, t_block] (moving 512) -> h1T/h3T.
  SwiGLU:  hT = silu(h1T) * h3T  (ACT Silu -> fp16, DVE mul -> fp16).
  mm2:     lhsT=hT[j_chunk, t_chunk] (stationary), rhs=w2[j_chunk, o_block]
           (moving 512) -> psum[t,o] natural-layout f32 output.
"""

import sys

sys.path.insert(0, "/opt/trn_rl_repo")

import numpy as np

N_CORES = 8
D = 1024  # dim_in
H = 1024  # dim_hid
P = 128
TB = 512  # token block per pipeline stage

_CACHE = {}


def _build(tok):
    import concourse.bacc as bacc
    import concourse.tile as tile
    from concourse import mybir
    from concourse.masks import make_identity

    dt = mybir.dt
    AF = mybir.ActivationFunctionType
    f32 = dt.float32
    f16 = dt.float16

    assert tok % TB == 0
    n_blk = tok // TB
    n_i = D // P   # 8 contraction chunks for mm1/mm3
    n_j = H // P   # 8 contraction chunks for mm2
    n_tc = TB // P  # 4 token chunks per block
    n_o = D // 512  # 2 output column blocks

    nc = bacc.Bacc(trn_type="TRN2", target_bir_lowering=False)
    x_h = nc.dram_tensor("x", [tok, D], f32, kind="ExternalInput")
    w1_h = nc.dram_tensor("w1", [D, H], f32, kind="ExternalInput")
    w2_h = nc.dram_tensor("w2", [H, D], f32, kind="ExternalInput")
    w3_h = nc.dram_tensor("w3", [D, H], f32, kind="ExternalInput")
    out_h = nc.dram_tensor("out", [tok, D], f32, kind="ExternalOutput")

    with tile.TileContext(nc) as tc:
        with (
            tc.tile_pool(name="wpool", bufs=1) as wpool,
            tc.tile_pool(name="const", bufs=1) as const,
            tc.tile_pool(name="xpool", bufs=3) as xpool,
            tc.tile_pool(name="xtpool", bufs=2) as xtpool,
            tc.tile_pool(name="htpool", bufs=2) as htpool,
            tc.tile_pool(name="spool", bufs=3) as spool,
            tc.tile_pool(name="opool", bufs=2) as opool,
            tc.tile_pool(name="pT", bufs=3, space="PSUM") as pTp,
            tc.tile_pool(name="pAB", bufs=3, space="PSUM") as pABp,
            tc.tile_pool(name="pC", bufs=2, space="PSUM") as pCp,
        ):
            ident = const.tile([P, P], f16)
            make_identity(nc, ident)

            x_r = x_h[:, :].rearrange("(b c p) d -> b p c d", p=P, c=n_tc)
            o_r = out_h[:, :].rearrange("(b c p) d -> b p c d", p=P, c=n_tc)

            # Pre-issue the first x blocks so the PE can start transposing
            # before the (large) weight DMAs hog the SWDGE queue.
            x16_pre = []
            for b in range(min(2, n_blk)):
                x16 = xpool.tile([P, n_tc, D], f16)
                nc.gpsimd.dma_start(out=x16, in_=x_r[b])
                x16_pre.append(x16)

            # Resident fp16 weights, cast in-flight by SWDGE cast-DMAs.
            # Layout: partition = row-within-chunk: [P, n_chunks, cols].
            w1s = wpool.tile([P, n_i, H], f16)
            w3s = wpool.tile([P, n_i, H], f16)
            w2s = wpool.tile([P, n_j, D], f16)
            nc.gpsimd.dma_start(
                out=w1s, in_=w1_h[:, :].rearrange("(c p) h -> p c h", p=P)
            )
            nc.gpsimd.dma_start(
                out=w3s, in_=w3_h[:, :].rearrange("(c p) h -> p c h", p=P)
            )
            nc.gpsimd.dma_start(
                out=w2s, in_=w2_h[:, :].rearrange("(c p) h -> p c h", p=P)
            )

            for b in range(n_blk):
                # ---- load + cast x block, natural layout [P, n_tc, D] fp16
                if b < len(x16_pre):
                    x16 = x16_pre[b]
                else:
                    x16 = xpool.tile([P, n_tc, D], f16)
                    nc.gpsimd.dma_start(out=x16, in_=x_r[b])

                # ---- PE-transpose into xT [P(=i in chunk), n_i, TB] fp16.
                # 8 transposes share one full PSUM bank, evicted by a single
                # wide copy (amortizes the ~240ns fixed ACT/DVE op cost),
                # alternating between ACT and DVE to double eviction rate.
                xT = xtpool.tile([P, n_i, TB], f16)
                for t in range(n_tc):
                    pTg = pTp.tile([P, n_i, P], f16)
                    for i in range(n_i):
                        nc.tensor.transpose(
                            pTg[:, i, :], x16[:, t, i * P:(i + 1) * P], ident
                        )
                    if t % 2 == 0:
                        nc.scalar.activation(
                            xT[:, :, t * P:(t + 1) * P], pTg, AF.Copy
                        )
                    else:
                        nc.vector.tensor_copy(
                            xT[:, :, t * P:(t + 1) * P], pTg
                        )

                # ---- mm1/mm3 + SwiGLU -> hT [P(=j in chunk), n_j, TB] fp16
                hT = htpool.tile([P, n_j, TB], f16)
                for j in range(n_j):
                    pA = pABp.tile([P, TB], f32, tag="pAB")
                    pB = pABp.tile([P, TB], f32, tag="pAB")
                    for i in range(n_i):
                        nc.tensor.matmul(
                            pA, w1s[:, i, j * P:(j + 1) * P], xT[:, i, :],
                            start=(i == 0), stop=(i == n_i - 1),
                        )
                    for i in range(n_i):
                        nc.tensor.matmul(
                            pB, w3s[:, i, j * P:(j + 1) * P], xT[:, i, :],
                            start=(i == 0), stop=(i == n_i - 1),
                        )
                    s1 = spool.tile([P, TB], f16)
                    nc.scalar.activation(s1, pA, AF.Silu)
                    nc.vector.tensor_mul(hT[:, j, :], pB, s1)

                # ---- mm2 -> natural-layout out block
                o_sb = opool.tile([P, n_tc, D], f32)
                for t in range(n_tc):
                    for o in range(n_o):
                        pC = pCp.tile([P, 512], f32)
                        for j in range(n_j):
                            nc.tensor.matmul(
                                pC,
                                hT[:, j, t * P:(t + 1) * P],
                                w2s[:, j, o * 512:(o + 1) * 512],
                                start=(j == 0), stop=(j == n_j - 1),
                            )
                        nc.scalar.activation(
                            o_sb[:, t, o * 512:(o + 1) * 512], pC, AF.Copy
                        )
                nc.sync.dma_start(out=o_r[b], in_=o_sb)

    nc.compile()
    return nc


def _get_nc(tok):
    if tok not in _CACHE:
        _CACHE[tok] = _build(tok)
    return _CACHE[tok]


def kernel(x, w1, w2, w3, m_sizes):
    from concourse.bass_utils import run_bass_kernel_spmd

    x = np.asarray(x, dtype=np.float32)
    w1 = np.asarray(w1, dtype=np.float32)
    w2 = np.asarray(w2, dtype=np.float32)
    w3 = np.asarray(w3, dtype=np.float32)
    sizes = np.asarray(m_sizes).astype(np.int64)
    offs = np.concatenate([[0], np.cumsum(sizes)])
    n_exp = sizes.shape[0]
    assert n_exp == N_CORES

    pad = int(max(int(sizes.max()), TB))
    pad = ((pad + TB - 1) // TB) * TB
    nc = _get_nc(pad)

    in_maps = []
    for e in range(N_CORES):
        xe = x[offs[e]:offs[e + 1]]
        if xe.shape[0] < pad:
            xe = np.concatenate(
                [xe, np.zeros((pad - xe.shape[0], D), dtype=np.float32)], axis=0
            )
        in_maps.append({"x": xe, "w1": w1[e], "w2": w2[e], "w3": w3[e]})

    r = run_bass_kernel_spmd(nc, in_maps, core_ids=list(range(N_CORES)))
    out = np.concatenate(
        [r.results[e]["out"][: sizes[e]] for e in range(N_CORES)], axis=0
    )
    return out.astype(np.float32)
